# revision 8
# baseline (speedup 1.0000x reference)
"""Trainium2 Bass kernel for a 4-layer Qwen3-style decoder + LM head.

Sharding: DP=2 over batch x TP=4 within each half (cores 0-3 = batch 0,
cores 4-7 = batch 1). Per core: 4 q-heads, 2 kv-heads, 768 ffn, 8000 vocab.
Two AllReduces of the residual stream per layer (attn out + mlp out), each
split into two 512-token halves for comm/compute overlap.

Layout: the residual stream h lives feature-major ([D, tokens]) in SBUF in
float32r so every GEMM consumes it directly (contraction dim on partitions).
RMS-norm reductions over the partition dim use ones-matmuls; the norm scale
s[t] is folded into GEMM outputs (it cancels exactly inside q/k head norms,
scales v / mlp / logits). QKV is produced token-major (easy head-RMS-norm /
RoPE / kv-cache output), then q/k are PE-transposed per head into
feature-major for attention, which computes scores^T [k, q] so the softmax
denominator folds into the output. All matmuls run in float32r (~1.5e-4).
"""

import numpy as np

# model dims (hardcoded from the problem spec)
L, D, HQ, HK, HD, FF, V = 4, 1024, 16, 8, 128, 3072, 32000
B, S = 2, 1024
EPS = 1e-6
THETA = 1000000.0
SCALE = HD ** -0.5

NCORES = 8
TP = 4            # tensor-parallel within a batch group
NQH = HQ // TP    # 4 q heads per core
NKV = HK // TP    # 2 kv heads per core
FFS = FF // TP    # 768
NJ = FFS // 128   # 6
VS = V // TP      # 8000 vocab per core
VC = 500          # vocab chunk (<=512 psum bank)
NVC = VS // VC    # 16
KT_D = D // 128   # 8 contraction tiles over D
TB = S // 128     # 8 token blocks per core (1024 tokens)
REPLICA_GROUPS = [[0, 1, 2, 3], [4, 5, 6, 7]]

_PROGRAM = None

# probe knobs (test-only; harness uses defaults)
CFG_LAYERS = L
CFG_COLLECTIVES = True
CFG_LM = True
CFG_ATTN = True


def _build_program():
    from contextlib import ExitStack
    import concourse.bass as bass
    import concourse.mybir as mybir
    import concourse.tile as tile
    from concourse import bacc
    from concourse.masks import make_identity, make_upper_triangular

    dt = mybir.dt
    f32, f32r = dt.float32, dt.float32r
    AF = mybir.ActivationFunctionType
    AX = mybir.AxisListType
    ALU = mybir.AluOpType

    nc = bacc.Bacc("TRN2", target_bir_lowering=False, debug=False,
                   num_devices=NCORES)

    x_in = nc.dram_tensor("x", [D, S], f32r, kind="ExternalInput")
    cos_in = nc.dram_tensor("cos", [S, HD], f32, kind="ExternalInput")
    sin_in = nc.dram_tensor("sin", [S, HD], f32, kind="ExternalInput")
    wqkv_in = nc.dram_tensor("wqkv", [L, D, 1024], f32r, kind="ExternalInput")
    wo_in = nc.dram_tensor("wo", [L, NQH * HD, D], f32r, kind="ExternalInput")
    wgu_in = nc.dram_tensor("wgu", [L, D, 2 * FFS], f32r, kind="ExternalInput")
    wd_in = nc.dram_tensor("wd", [L, FFS, D], f32r, kind="ExternalInput")
    lmw_in = nc.dram_tensor("lmw", [D, VS], f32r, kind="ExternalInput")

    logits_out = nc.dram_tensor("logits", [128, TB, VS], f32, kind="ExternalOutput")
    pk_out = nc.dram_tensor("pk", [L, NKV, S, HD], f32, kind="ExternalOutput")
    pv_out = nc.dram_tensor("pv", [L, NKV, S, HD], f32r, kind="ExternalOutput")

    def kt_view(ap2d):  # [D, N] dram AP -> [128, D//128, N]
        return ap2d.rearrange("(kt p) n -> p kt n", p=128)

    def tok_view(ap2d):  # [S, N] dram AP -> [128, S//128, N] (p = t % 128)
        return ap2d.rearrange("(tb p) n -> p tb n", p=128)

    with tile.TileContext(nc) as tc, ExitStack() as ctx:
        ent = ctx.enter_context
        ent(nc.allow_low_precision(reason="fp32r rounding intended"))
        resid_p = ent(tc.tile_pool(name="resid", bufs=1))
        wpool = ent(tc.tile_pool(name="wpool", bufs=2))
        qb_p = ent(tc.tile_pool(name="qb", bufs=1))     # QT | m_sb
        ot_p = ent(tc.tile_pool(name="ot", bufs=1))     # q_tok | oT | nT2
        ktok_p = ent(tc.tile_pool(name="ktok", bufs=1))
        vtok_p = ent(tc.tile_pool(name="vtok", bufs=1))
        ktp_p = ent(tc.tile_pool(name="ktp", bufs=1))
        cs_p = ent(tc.tile_pool(name="cs", bufs=1))
        const_p = ent(tc.tile_pool(name="const", bufs=1))
        scr_p = ent(tc.tile_pool(name="scr", bufs=3))
        expp = ent(tc.tile_pool(name="expp", bufs=3))
        rowp = ent(tc.tile_pool(name="rowp", bufs=1))
        mis_p = ent(tc.tile_pool(name="mis", bufs=3))
        evp = ent(tc.tile_pool(name="evp", bufs=3))
        tiny_p = ent(tc.tile_pool(name="tiny", bufs=2))
        ps_a = ent(tc.tile_pool(name="ps_a", bufs=5, space="PSUM"))
        ps_o = ent(tc.tile_pool(name="ps_o", bufs=1, space="PSUM"))
        ps_r = ent(tc.tile_pool(name="ps_r", bufs=2, space="PSUM"))
        dram_p = ent(tc.tile_pool(name="dram", bufs=3, space="DRAM"))

        # ---- constants: one packed f32 tile + one all-ones f32r tile
        cf = const_p.tile([128, 258], f32)
        ident = cf[:, 0:128]
        tri = cf[:, 128:256]          # 1 where k<=q (row<=col)
        eps_c = cf[:, 256:257]
        ones_f = cf[:, 257:258]
        make_identity(nc, ident)
        make_upper_triangular(nc, tri, val=1.0, diag=True)
        nc.vector.memset(eps_c, EPS)
        nc.vector.memset(ones_f, 1.0)
        ones = const_p.tile([128, 128], f32r)
        ones_b = bass.AP(tensor=ones_f.tensor, offset=ones_f.offset,
                         ap=[list(ones_f.ap[0]), [0, 128]])
        nc.vector.tensor_copy(out=ones[:], in_=ones_b)
        ones_col = ones[:, 0:1]       # [128, 1]
        ones_row = ones[0:1, :]       # [1, 128]

        # ---- residual stream + rope tables
        hT = resid_p.tile([128, KT_D, S], f32r)
        for qt in range(2):
            nc.sync.dma_start(out=hT[:, :, qt * 512:(qt + 1) * 512],
                              in_=kt_view(x_in[:])[:, :, qt * 512:(qt + 1) * 512])
        cos_sb = cs_p.tile([128, TB, HD], f32)
        sin_sb = cs_p.tile([128, TB, HD], f32)
        nc.sync.dma_start(out=cos_sb[:], in_=tok_view(cos_in[:]))
        nc.sync.dma_start(out=sin_sb[:], in_=tok_view(sin_in[:]))

        def ln_scale(qt):
            """Return S_ps psum [128, 512]: rows all equal the rms-norm scale
            s[t] for tokens [qt*512, qt*512+512) of hT."""
            t0 = qt * 512
            ss = ps_r.tile([1, 512], f32, tag="row")
            for ki in range(KT_D):
                sq = scr_p.tile([128, 512], f32r, tag="sq")
                hs = hT[:, ki, t0:t0 + 512]
                if ki % 2 == 0:
                    nc.scalar.activation(out=sq[:], in_=hs, func=AF.Square)
                else:
                    nc.vector.tensor_mul(sq[:], hs, hs)
                nc.tensor.matmul(ss[:], ones_col, sq[:],
                                 start=(ki == 0), stop=(ki == KT_D - 1))
            sr = rowp.tile([1, 512], f32, tag="sr")
            nc.scalar.activation(out=sr[:], in_=ss[:], func=AF.Sqrt,
                                 scale=1.0 / D, bias=eps_c[0:1, :])
            s_row = rowp.tile([1, 512], f32r, tag="srow")
            nc.vector.reciprocal(out=s_row[:], in_=sr[:])
            S_ps = ps_a.tile([128, 512], f32, tag="mm")
            nc.tensor.matmul(S_ps[:], ones_row, s_row[:], start=True, stop=True)
            return S_ps

        def diag_stok(S_ps, stok, col0):
            """stok[:, i:i+1] = diag(S_ps[:, col0:col0+128]) = s per token."""
            dg = mis_p.tile([128, 128], f32, tag="dg")
            nc.vector.tensor_mul(dg[:], S_ps[:, col0:col0 + 128], ident)
            nc.vector.tensor_reduce(stok, dg[:], axis=AX.X, op=ALU.add)

        def bcast_heads(t2d, nh):
            return bass.AP(tensor=t2d.tensor, offset=t2d.offset,
                           ap=[list(t2d.ap[0]), [0, nh], list(t2d.ap[1])])

        def rope_inplace(tokv, rot, cs2d, sn2d, nh):
            """tokv: [128, nh, 128] AP (token-major head data), in-place rope."""
            cb = bcast_heads(cs2d, nh)
            sb = bcast_heads(sn2d, nh)
            h2 = HD // 2
            nc.vector.tensor_scalar_mul(rot[:, :, 0:h2], tokv[:, :, h2:HD], -1.0)
            nc.vector.tensor_copy(out=rot[:, :, h2:HD], in_=tokv[:, :, 0:h2])
            nc.vector.tensor_mul(tokv, tokv, cb)
            nc.vector.tensor_mul(rot[:], rot[:], sb)
            nc.vector.tensor_add(tokv, tokv, rot[:])

        def headnorm(ps, c0, nh, t6, o0, tag):
            """rms-normalize nh heads of token-major psum ps[:, c0:c0+nh*128];
            returns [128, nh] reciprocal-rms AP (slice o0 of t6)."""
            sq3 = mis_p.tile([128, nh, HD], f32, tag=tag)
            nc.scalar.activation(
                out=sq3[:], func=AF.Square,
                in_=ps[:, c0:c0 + nh * HD].rearrange("p (h d) -> p h d", h=nh))
            r0 = t6[:, o0:o0 + nh]
            nc.vector.tensor_reduce(r0, sq3[:], axis=AX.X, op=ALU.add)
            r1 = t6[:, o0 + nh:o0 + 2 * nh]
            nc.scalar.activation(out=r1, in_=r0, func=AF.Sqrt,
                                 scale=1.0 / HD, bias=eps_c[:, :])
            r2 = t6[:, o0 + 2 * nh:o0 + 3 * nh]
            nc.vector.reciprocal(out=r2, in_=r1)
            return r2

        def allreduce_half(src_fn, qt):
            """src_fn(dm) -> psum tile [128,512]; AR it into hT half qt."""
            t0 = qt * 512
            arin = dram_p.tile([128, KT_D, 512], f32, tag="arin")
            arout = dram_p.tile([128, KT_D, 512], f32, tag="arout")
            for dm in range(KT_D):
                pp = src_fn(dm)
                ev = evp.tile([128, 512], f32, tag="ev")
                nc.scalar.copy(out=ev[:], in_=pp[:])
                nc.sync.dma_start(out=arin[:, dm, :], in_=ev[:])
            if CFG_COLLECTIVES:
                nc.gpsimd.collective_compute(
                    "AllReduce", ALU.add, replica_groups=REPLICA_GROUPS,
                    ins=[arin[:].opt()], outs=[arout[:].opt()])
            else:
                nc.sync.dma_start(out=arout[:], in_=arin[:])
            for dm in range(KT_D):
                rb = evp.tile([128, 512], f32, tag="rb")
                nc.sync.dma_start(out=rb, in_=arout[:, dm, :])
                nc.vector.tensor_add(hT[:, dm, t0:t0 + 512],
                                     hT[:, dm, t0:t0 + 512], rb[:])

        for l in range(CFG_LAYERS):
            # ---------- QKV (ln1 scale folds: cancels in q/k norms, scales v)
            wq_c = []
            for c in range(2):
                w = wpool.tile([128, KT_D, 512], f32r, tag="w")
                nc.sync.dma_start(out=w, in_=kt_view(wqkv_in[l])[:, :, c * 512:(c + 1) * 512])
                wq_c.append(w)
            q_tok = ot_p.tile([128, TB, NQH * HD], f32, tag="ot")
            k_tok = ktok_p.tile([128, TB, NKV * HD], f32)
            v_sb = vtok_p.tile([128, TB, NKV * HD], f32r)

            for qt in range(2):
                S_ps = ln_scale(qt)
                stok = tiny_p.tile([128, 4], f32, tag="stok")
                for i in range(4):
                    diag_stok(S_ps, stok[:, i:i + 1], i * 128)
                for tb in range(qt * 4, qt * 4 + 4):
                    ps_q = ps_a.tile([128, 512], f32, tag="mm")
                    ps_kv = ps_a.tile([128, 512], f32, tag="mm")
                    for ki in range(KT_D):
                        st, sp = (ki == 0), (ki == KT_D - 1)
                        lhs = hT[:, ki, tb * 128:(tb + 1) * 128]
                        nc.tensor.matmul(ps_q[:], lhs, wq_c[0][:, ki, :], start=st, stop=sp)
                        nc.tensor.matmul(ps_kv[:], lhs, wq_c[1][:, ki, :], start=st, stop=sp)
                    t6 = tiny_p.tile([128, 18], f32, tag="t6")
                    rqi = headnorm(ps_q, 0, NQH, t6, 0, "m4")
                    for h in range(NQH):
                        nc.vector.tensor_scalar_mul(q_tok[:, tb, h * HD:(h + 1) * HD],
                                                    ps_q[:, h * HD:(h + 1) * HD],
                                                    rqi[:, h:h + 1])
                    rki = headnorm(ps_kv, 0, NKV, t6, 12, "m2")
                    for h in range(NKV):
                        nc.vector.tensor_scalar_mul(k_tok[:, tb, h * HD:(h + 1) * HD],
                                                    ps_kv[:, h * HD:(h + 1) * HD],
                                                    rki[:, h:h + 1])
                    # v evict with ln1 scale (per-token = per-partition here)
                    nc.vector.tensor_scalar_mul(v_sb[:, tb, :], ps_kv[:, 256:512],
                                                stok[:, tb - qt * 4:tb - qt * 4 + 1])
                    # rope q and k in place
                    rotq = mis_p.tile([128, NQH, HD], f32, tag="m4")
                    rope_inplace(q_tok[:, tb, :].rearrange("p (h d) -> p h d", h=NQH),
                                 rotq, cos_sb[:, tb, :], sin_sb[:, tb, :], NQH)
                    rotk = mis_p.tile([128, NKV, HD], f32, tag="m2")
                    rope_inplace(k_tok[:, tb, :].rearrange("p (h d) -> p h d", h=NKV),
                                 rotk, cos_sb[:, tb, :], sin_sb[:, tb, :], NKV)

            # kv cache outputs
            for kv in range(NKV):
                nc.sync.dma_start(out=tok_view(pk_out[l, kv]),
                                  in_=k_tok[:, :, kv * HD:(kv + 1) * HD])
                nc.sync.dma_start(out=tok_view(pv_out[l, kv]),
                                  in_=v_sb[:, :, kv * HD:(kv + 1) * HD])

            # ---------- transposes to feature-major
            QT = qb_p.tile([128, NQH, S], f32r, tag="qb")
            KT = ktp_p.tile([128, NKV, S], f32r)
            for h in range(NQH):
                for tb in range(TB):
                    tps = ps_a.tile([128, 128], f32, tag="mm")
                    nc.tensor.transpose(tps[:], q_tok[:, tb, h * HD:(h + 1) * HD], ident)
                    nc.vector.tensor_copy(out=QT[:, h, tb * 128:(tb + 1) * 128], in_=tps[:])
            for h in range(NKV):
                for tb in range(TB):
                    tps = ps_a.tile([128, 128], f32, tag="mm")
                    nc.tensor.transpose(tps[:], k_tok[:, tb, h * HD:(h + 1) * HD], ident)
                    nc.vector.tensor_copy(out=KT[:, h, tb * 128:(tb + 1) * 128], in_=tps[:])

            # ---------- attention (scores^T layout, causal skip)
            oT = ot_p.tile([128, NQH, S], f32r, tag="ot")
            for kv in range(NKV if CFG_ATTN else 0):
                for hh in range(2):
                    qh = kv * 2 + hh
                    for qt in range(2):
                        o_ps = ps_o.tile([128, 512], f32, tag="o")
                        dn_ps = ps_r.tile([1, 512], f32, tag="row")
                        nkb = qt * 4 + 4
                        for kb in range(nkb):
                            off = max(0, kb * 128 - qt * 512)
                            w = 512 - off
                            sc = ps_a.tile([128, 512], f32, tag="mm")
                            nc.tensor.matmul(sc[:, :w], KT[:, kv, kb * 128:(kb + 1) * 128],
                                             QT[:, qh, qt * 512 + off:(qt + 1) * 512],
                                             start=True, stop=True)
                            ex = expp.tile([128, 512], f32r, tag="exp")
                            nc.scalar.activation(out=ex[:, :w], in_=sc[:, :w],
                                                 func=AF.Exp, scale=SCALE)
                            if kb * 128 >= qt * 512:  # diagonal block
                                nc.vector.tensor_mul(ex[:, 0:128], ex[:, 0:128], tri)
                            st, sp = (kb == 0), (kb == nkb - 1)
                            nc.tensor.matmul(o_ps[:, off:512],
                                             v_sb[:, kb, kv * HD:(kv + 1) * HD],
                                             ex[:, :w], start=st, stop=sp)
                            nc.tensor.matmul(dn_ps[:, off:512], ones_col,
                                             ex[:, :w], start=st, stop=sp)
                        dinv = rowp.tile([1, 512], f32r, tag="dinv")
                        nc.vector.reciprocal(out=dinv[:], in_=dn_ps[:])
                        bc = ps_a.tile([128, 512], f32, tag="mm")
                        nc.tensor.matmul(bc[:], ones_row, dinv[:], start=True, stop=True)
                        dful = mis_p.tile([128, 512], f32, tag="d5")
                        nc.scalar.copy(out=dful[:], in_=bc[:])
                        nc.vector.tensor_mul(oT[:, qh, qt * 512:(qt + 1) * 512],
                                             o_ps[:], dful[:])

            # ---------- o_proj + AllReduce
            wo_c = []
            for c in range(2):
                w = wpool.tile([128, NQH, 512], f32r, tag="w")
                nc.sync.dma_start(out=w, in_=kt_view(wo_in[l])[:, :, c * 512:(c + 1) * 512])
                wo_c.append(w)
            for qt in range(2):
                def oproj_dm(dm, qt=qt):
                    pp = ps_a.tile([128, 512], f32, tag="mm")
                    for ki in range(NQH):
                        nc.tensor.matmul(pp[:], wo_c[dm // 4][:, ki, (dm % 4) * 128:(dm % 4 + 1) * 128],
                                         oT[:, ki, qt * 512:(qt + 1) * 512],
                                         start=(ki == 0), stop=(ki == NQH - 1))
                    return pp
                allreduce_half(oproj_dm, qt)

            # ---------- MLP (ln2 applied to nT2; gate/up/down GEMMs)
            wgu_c = []
            for c in range(3):
                w = wpool.tile([128, KT_D, 512], f32r, tag="w")
                nc.sync.dma_start(out=w, in_=kt_view(wgu_in[l])[:, :, c * 512:(c + 1) * 512])
                wgu_c.append(w)
            wd_c = []
            for c in range(2):
                w = wpool.tile([128, NJ, 512], f32r, tag="w")
                nc.sync.dma_start(out=w, in_=kt_view(wd_in[l])[:, :, c * 512:(c + 1) * 512])
                wd_c.append(w)
            nT2 = ot_p.tile([128, KT_D, S], f32r, tag="ot")
            m_sb = qb_p.tile([128, NJ, S], f32r, tag="qb")
            for qt in range(2):
                t0 = qt * 512
                S_ps = ln_scale(qt)
                for ki in range(KT_D):
                    nc.vector.tensor_mul(nT2[:, ki, t0:t0 + 512],
                                         hT[:, ki, t0:t0 + 512], S_ps[:])
            # wgu is host-interleaved: chunk j//2 holds [gate_j, up_j, gate_j+1, up_j+1]
            for j in range(NJ):
                gc = (2 * (j % 2)) * 128
                uc = (2 * (j % 2) + 1) * 128
                for qt in range(2):
                    t0 = qt * 512
                    gp = ps_a.tile([128, 512], f32, tag="mm")
                    for ki in range(KT_D):
                        nc.tensor.matmul(gp[:], wgu_c[j // 2][:, ki, gc:gc + 128],
                                         nT2[:, ki, t0:t0 + 512],
                                         start=(ki == 0), stop=(ki == KT_D - 1))
                    g_t = mis_p.tile([128, 512], f32, tag="d5")
                    nc.scalar.activation(out=g_t[:], in_=gp[:], func=AF.Silu)
                    up = ps_a.tile([128, 512], f32, tag="mm")
                    for ki in range(KT_D):
                        nc.tensor.matmul(up[:], wgu_c[j // 2][:, ki, uc:uc + 128],
                                         nT2[:, ki, t0:t0 + 512],
                                         start=(ki == 0), stop=(ki == KT_D - 1))
                    nc.vector.tensor_mul(m_sb[:, j, t0:t0 + 512], up[:], g_t[:])
            for qt in range(2):
                def down_dm(dm, qt=qt):
                    pp = ps_a.tile([128, 512], f32, tag="mm")
                    for ki in range(NJ):
                        nc.tensor.matmul(pp[:], wd_c[dm // 4][:, ki, (dm % 4) * 128:(dm % 4 + 1) * 128],
                                         m_sb[:, ki, qt * 512:(qt + 1) * 512],
                                         start=(ki == 0), stop=(ki == NJ - 1))
                    return pp
                allreduce_half(down_dm, qt)

        # ---------- final norm (folded into logit eviction) + lm head
        stokf = tiny_p.tile([128, TB], f32, tag="stokf")
        for qt in range(2):
            S_ps = ln_scale(qt)
            for i in range(4):
                diag_stok(S_ps, stokf[:, qt * 4 + i:qt * 4 + i + 1], i * 128)
        for vc in range(NVC if CFG_LM else 0):
            lw = wpool.tile([128, KT_D, VC], f32r, tag="w")
            nc.sync.dma_start(out=lw, in_=kt_view(lmw_in[:])[:, :, vc * VC:(vc + 1) * VC])
            for tb in range(TB):
                pp = ps_a.tile([128, 512], f32, tag="mm")
                for ki in range(KT_D):
                    nc.tensor.matmul(pp[:, :VC], hT[:, ki, tb * 128:(tb + 1) * 128],
                                     lw[:, ki, :], start=(ki == 0), stop=(ki == KT_D - 1))
                ev = evp.tile([128, 512], f32, tag="ev")
                nc.vector.tensor_scalar_mul(ev[:, :VC], pp[:, :VC], stokf[:, tb:tb + 1])
                nc.sync.dma_start(out=logits_out[:, tb, vc * VC:(vc + 1) * VC],
                                  in_=ev[:, :VC])

    nc.compile()
    return nc


def _get_program():
    global _PROGRAM
    if _PROGRAM is None:
        _PROGRAM = _build_program()
    return _PROGRAM


def _host_prep(inputs):
    """Build the 8 per-core input maps from the full inputs."""
    pos = np.asarray(inputs["position_ids"], np.float32)  # [B, S]
    inv_freq = 1.0 / (THETA ** (np.arange(0, HD, 2, dtype=np.float32) / HD))
    in_maps = []
    for c in range(NCORES):
        dp, tp = c // TP, c % TP
        freqs = pos[dp][:, None] * inv_freq[None, :]        # [S, 64]
        emb = np.concatenate([freqs, freqs], axis=1)        # [S, 128]
        wq = np.asarray(inputs["Wq"])[:, :, tp * NQH * HD:(tp + 1) * NQH * HD]
        wk = np.asarray(inputs["Wk"])[:, :, tp * NKV * HD:(tp + 1) * NKV * HD]
        wv = np.asarray(inputs["Wv"])[:, :, tp * NKV * HD:(tp + 1) * NKV * HD]
        # fold ln weights into the in-projections (all-ones here, but general)
        ln1 = np.asarray(inputs["ln1_w"])[:, :, None]       # [L, D, 1]
        ln2 = np.asarray(inputs["ln2_w"])[:, :, None]
        lnf = np.asarray(inputs["norm_w"])[:, None]         # [D, 1]
        wqkv = np.concatenate([wq, wk, wv], axis=2) * ln1   # [L, D, 1024]
        in_maps.append({
            "x": np.ascontiguousarray(np.asarray(inputs["input_embeds"])[dp].T),
            "cos": np.cos(emb).astype(np.float32),
            "sin": np.sin(emb).astype(np.float32),
            "wqkv": np.ascontiguousarray(wqkv, np.float32),
            "wo": np.ascontiguousarray(
                np.asarray(inputs["Wo"])[:, tp * NQH * HD:(tp + 1) * NQH * HD, :], np.float32),
            "wgu": np.ascontiguousarray((np.stack(
                [np.asarray(inputs["Wgate"])[:, :, tp * FFS:(tp + 1) * FFS]
                   .reshape(L, D, NJ, 128),
                 np.asarray(inputs["Wup"])[:, :, tp * FFS:(tp + 1) * FFS]
                   .reshape(L, D, NJ, 128)],
                axis=3).reshape(L, D, 2 * FFS)) * ln2, np.float32),
            "wd": np.ascontiguousarray(
                np.asarray(inputs["Wdown"])[:, tp * FFS:(tp + 1) * FFS, :], np.float32),
            "lmw": np.ascontiguousarray(
                (np.asarray(inputs["lm_head_w"]) * lnf)[:, tp * VS:(tp + 1) * VS], np.float32),
        })
    return in_maps


def _assemble(results):
    logits = np.empty((B, S, V), np.float32)
    pk = np.empty((L, B, HK, S, HD), np.float32)
    pv = np.empty((L, B, HK, S, HD), np.float32)
    for c in range(NCORES):
        dp, tp = c // TP, c % TP
        r = results[c]
        lg = r["logits"]  # [128, TB, VS]
        logits[dp, :, tp * VS:(tp + 1) * VS] = lg.transpose(1, 0, 2).reshape(S, VS)
        pk[:, dp, tp * NKV:(tp + 1) * NKV] = r["pk"]
        pv[:, dp, tp * NKV:(tp + 1) * NKV] = r["pv"]
    return logits, pk, pv


def kernel(**inputs):
    from concourse.bass_utils import run_bass_kernel_spmd
    nc = _get_program()
    in_maps = _host_prep(inputs)
    res = run_bass_kernel_spmd(nc, in_maps, core_ids=list(range(NCORES)))
    return _assemble(res.results)


# revision 9
# speedup vs baseline: 1.0117x; 1.0117x over previous
"""Trainium2 Bass kernel for a 4-layer Qwen3-style decoder + LM head.

Sharding: DP=2 over batch x TP=4 within each half (cores 0-3 = batch 0,
cores 4-7 = batch 1). Per core: 4 q-heads, 2 kv-heads, 768 ffn, 8000 vocab.
Two AllReduces of the residual stream per layer (attn out + mlp out), each
split into two 512-token halves for comm/compute overlap.

Layout: the residual stream h lives feature-major ([D, tokens]) in SBUF in
float32r so every GEMM consumes it directly (contraction dim on partitions).
RMS-norm reductions over the partition dim use ones-matmuls; the norm scale
s[t] is folded into GEMM outputs (it cancels exactly inside q/k head norms,
scales v / mlp / logits). QKV is produced token-major (easy head-RMS-norm /
RoPE / kv-cache output), then q/k are PE-transposed per head into
feature-major for attention, which computes scores^T [k, q] so the softmax
denominator folds into the output. All matmuls run in float32r (~1.5e-4).
"""

import numpy as np

# model dims (hardcoded from the problem spec)
L, D, HQ, HK, HD, FF, V = 4, 1024, 16, 8, 128, 3072, 32000
B, S = 2, 1024
EPS = 1e-6
THETA = 1000000.0
SCALE = HD ** -0.5

NCORES = 8
TP = 4            # tensor-parallel within a batch group
NQH = HQ // TP    # 4 q heads per core
NKV = HK // TP    # 2 kv heads per core
FFS = FF // TP    # 768
NJ = FFS // 128   # 6
VS = V // TP      # 8000 vocab per core
VC = 500          # vocab chunk (<=512 psum bank)
NVC = VS // VC    # 16
KT_D = D // 128   # 8 contraction tiles over D
TB = S // 128     # 8 token blocks per core (1024 tokens)
REPLICA_GROUPS = [[0, 1, 2, 3], [4, 5, 6, 7]]

_PROGRAM = None

# probe knobs (test-only; harness uses defaults)
CFG_LAYERS = L
CFG_COLLECTIVES = True
CFG_LM = True
CFG_ATTN = True


def _build_program():
    from contextlib import ExitStack
    import concourse.bass as bass
    import concourse.mybir as mybir
    import concourse.tile as tile
    from concourse import bacc
    from concourse.masks import make_identity, make_upper_triangular

    dt = mybir.dt
    f32, f32r = dt.float32, dt.float32r
    AF = mybir.ActivationFunctionType
    AX = mybir.AxisListType
    ALU = mybir.AluOpType

    nc = bacc.Bacc("TRN2", target_bir_lowering=False, debug=False,
                   num_devices=NCORES)

    x_in = nc.dram_tensor("x", [D, S], f32r, kind="ExternalInput")
    cos_in = nc.dram_tensor("cos", [S, HD], f32, kind="ExternalInput")
    sin_in = nc.dram_tensor("sin", [S, HD], f32, kind="ExternalInput")
    wqkv_in = nc.dram_tensor("wqkv", [L, D, 1024], f32r, kind="ExternalInput")
    wo_in = nc.dram_tensor("wo", [L, NQH * HD, D], f32r, kind="ExternalInput")
    wgu_in = nc.dram_tensor("wgu", [L, D, 2 * FFS], f32r, kind="ExternalInput")
    wd_in = nc.dram_tensor("wd", [L, FFS, D], f32r, kind="ExternalInput")
    lmw_in = nc.dram_tensor("lmw", [D, VS], f32r, kind="ExternalInput")

    logits_out = nc.dram_tensor("logits", [128, TB, VS], f32, kind="ExternalOutput")
    pk_out = nc.dram_tensor("pk", [L, NKV, S, HD], f32, kind="ExternalOutput")
    pv_out = nc.dram_tensor("pv", [L, NKV, S, HD], f32r, kind="ExternalOutput")

    def kt_view(ap2d):  # [D, N] dram AP -> [128, D//128, N]
        return ap2d.rearrange("(kt p) n -> p kt n", p=128)

    def tok_view(ap2d):  # [S, N] dram AP -> [128, S//128, N] (p = t % 128)
        return ap2d.rearrange("(tb p) n -> p tb n", p=128)

    with tile.TileContext(nc) as tc, ExitStack() as ctx:
        ent = ctx.enter_context
        ent(nc.allow_low_precision(reason="fp32r rounding intended"))
        resid_p = ent(tc.tile_pool(name="resid", bufs=1))
        wpool = ent(tc.tile_pool(name="wpool", bufs=2))
        qb_p = ent(tc.tile_pool(name="qb", bufs=1))     # QT | m_sb
        ot_p = ent(tc.tile_pool(name="ot", bufs=1))     # q_tok | oT | nT2
        ktok_p = ent(tc.tile_pool(name="ktok", bufs=1))
        vtok_p = ent(tc.tile_pool(name="vtok", bufs=1))
        ktp_p = ent(tc.tile_pool(name="ktp", bufs=1))
        cs_p = ent(tc.tile_pool(name="cs", bufs=1))
        const_p = ent(tc.tile_pool(name="const", bufs=1))
        scr_p = ent(tc.tile_pool(name="scr", bufs=3))
        expp = ent(tc.tile_pool(name="expp", bufs=3))
        rowp = ent(tc.tile_pool(name="rowp", bufs=1))
        mis_p = ent(tc.tile_pool(name="mis", bufs=3))
        evp = ent(tc.tile_pool(name="evp", bufs=3))
        tiny_p = ent(tc.tile_pool(name="tiny", bufs=2))
        ps_a = ent(tc.tile_pool(name="ps_a", bufs=4, space="PSUM"))
        ps_o = ent(tc.tile_pool(name="ps_o", bufs=2, space="PSUM"))
        ps_r = ent(tc.tile_pool(name="ps_r", bufs=2, space="PSUM"))
        dram_p = ent(tc.tile_pool(name="dram", bufs=3, space="DRAM"))

        # ---- constants: one packed f32 tile + one all-ones f32r tile
        cf = const_p.tile([128, 258], f32)
        ident = cf[:, 0:128]
        tri = cf[:, 128:256]          # 1 where k<=q (row<=col)
        eps_c = cf[:, 256:257]
        ones_f = cf[:, 257:258]
        make_identity(nc, ident)
        make_upper_triangular(nc, tri, val=1.0, diag=True)
        nc.vector.memset(eps_c, EPS)
        nc.vector.memset(ones_f, 1.0)
        ones = const_p.tile([128, 128], f32r)
        ones_b = bass.AP(tensor=ones_f.tensor, offset=ones_f.offset,
                         ap=[list(ones_f.ap[0]), [0, 128]])
        nc.vector.tensor_copy(out=ones[:], in_=ones_b)
        ones_col = ones[:, 0:1]       # [128, 1]
        ones_row = ones[0:1, :]       # [1, 128]

        # ---- residual stream + rope tables
        hT = resid_p.tile([128, KT_D, S], f32r)
        for qt in range(2):
            nc.sync.dma_start(out=hT[:, :, qt * 512:(qt + 1) * 512],
                              in_=kt_view(x_in[:])[:, :, qt * 512:(qt + 1) * 512])
        cos_sb = cs_p.tile([128, TB, HD], f32)
        sin_sb = cs_p.tile([128, TB, HD], f32)
        nc.sync.dma_start(out=cos_sb[:], in_=tok_view(cos_in[:]))
        nc.sync.dma_start(out=sin_sb[:], in_=tok_view(sin_in[:]))

        def ln_scale(qt):
            """Return S_ps psum [128, 512]: rows all equal the rms-norm scale
            s[t] for tokens [qt*512, qt*512+512) of hT."""
            t0 = qt * 512
            ss = ps_r.tile([1, 512], f32, tag="row")
            for ki in range(KT_D):
                sq = scr_p.tile([128, 512], f32r, tag="sq")
                hs = hT[:, ki, t0:t0 + 512]
                if ki % 2 == 0:
                    nc.scalar.activation(out=sq[:], in_=hs, func=AF.Square)
                else:
                    nc.vector.tensor_mul(sq[:], hs, hs)
                nc.tensor.matmul(ss[:], ones_col, sq[:],
                                 start=(ki == 0), stop=(ki == KT_D - 1))
            sr = rowp.tile([1, 512], f32, tag="sr")
            nc.scalar.activation(out=sr[:], in_=ss[:], func=AF.Sqrt,
                                 scale=1.0 / D, bias=eps_c[0:1, :])
            s_row = rowp.tile([1, 512], f32r, tag="srow")
            nc.vector.reciprocal(out=s_row[:], in_=sr[:])
            S_ps = ps_a.tile([128, 512], f32, tag="mm")
            nc.tensor.matmul(S_ps[:], ones_row, s_row[:], start=True, stop=True)
            return S_ps

        def diag_stok(S_ps, stok, col0):
            """stok[:, i:i+1] = diag(S_ps[:, col0:col0+128]) = s per token."""
            dg = mis_p.tile([128, 128], f32, tag="dg")
            nc.vector.tensor_mul(dg[:], S_ps[:, col0:col0 + 128], ident)
            nc.vector.tensor_reduce(stok, dg[:], axis=AX.X, op=ALU.add)

        def bcast_heads(t2d, nh):
            return bass.AP(tensor=t2d.tensor, offset=t2d.offset,
                           ap=[list(t2d.ap[0]), [0, nh], list(t2d.ap[1])])

        def rope_inplace(tokv, rot, cs2d, sn2d, nh):
            """tokv: [128, nh, 128] AP (token-major head data), in-place rope."""
            cb = bcast_heads(cs2d, nh)
            sb = bcast_heads(sn2d, nh)
            h2 = HD // 2
            nc.vector.tensor_scalar_mul(rot[:, :, 0:h2], tokv[:, :, h2:HD], -1.0)
            nc.vector.tensor_copy(out=rot[:, :, h2:HD], in_=tokv[:, :, 0:h2])
            nc.vector.tensor_mul(tokv, tokv, cb)
            nc.vector.tensor_mul(rot[:], rot[:], sb)
            nc.vector.tensor_add(tokv, tokv, rot[:])

        def headnorm(ps, c0, nh, t6, o0, tag):
            """rms-normalize nh heads of token-major psum ps[:, c0:c0+nh*128];
            returns [128, nh] reciprocal-rms AP (slice o0 of t6)."""
            sq3 = mis_p.tile([128, nh, HD], f32, tag=tag)
            nc.scalar.activation(
                out=sq3[:], func=AF.Square,
                in_=ps[:, c0:c0 + nh * HD].rearrange("p (h d) -> p h d", h=nh))
            r0 = t6[:, o0:o0 + nh]
            nc.vector.tensor_reduce(r0, sq3[:], axis=AX.X, op=ALU.add)
            r1 = t6[:, o0 + nh:o0 + 2 * nh]
            nc.scalar.activation(out=r1, in_=r0, func=AF.Sqrt,
                                 scale=1.0 / HD, bias=eps_c[:, :])
            r2 = t6[:, o0 + 2 * nh:o0 + 3 * nh]
            nc.vector.reciprocal(out=r2, in_=r1)
            return r2

        def allreduce_half(src_fn, qt):
            """src_fn(dm) -> psum tile [128,512]; AR it into hT half qt."""
            t0 = qt * 512
            arin = dram_p.tile([128, KT_D, 512], f32, tag="arin")
            arout = dram_p.tile([128, KT_D, 512], f32, tag="arout")
            for dm in range(KT_D):
                pp = src_fn(dm)
                ev = evp.tile([128, 512], f32, tag="ev")
                nc.scalar.copy(out=ev[:], in_=pp[:])
                nc.sync.dma_start(out=arin[:, dm, :], in_=ev[:])
            if CFG_COLLECTIVES:
                nc.gpsimd.collective_compute(
                    "AllReduce", ALU.add, replica_groups=REPLICA_GROUPS,
                    ins=[arin[:].opt()], outs=[arout[:].opt()])
            else:
                nc.sync.dma_start(out=arout[:], in_=arin[:])
            for dm in range(KT_D):
                rb = evp.tile([128, 512], f32, tag="rb")
                nc.sync.dma_start(out=rb, in_=arout[:, dm, :])
                nc.vector.tensor_add(hT[:, dm, t0:t0 + 512],
                                     hT[:, dm, t0:t0 + 512], rb[:])

        for l in range(CFG_LAYERS):
            # ---------- QKV (ln1 scale folds: cancels in q/k norms, scales v)
            wq_c = []
            for c in range(2):
                w = wpool.tile([128, KT_D, 512], f32r, tag="w")
                nc.sync.dma_start(out=w, in_=kt_view(wqkv_in[l])[:, :, c * 512:(c + 1) * 512])
                wq_c.append(w)
            q_tok = ot_p.tile([128, TB, NQH * HD], f32, tag="ot")
            k_tok = ktok_p.tile([128, TB, NKV * HD], f32)
            v_sb = vtok_p.tile([128, TB, NKV * HD], f32r)

            for qt in range(2):
                S_ps = ln_scale(qt)
                stok = tiny_p.tile([128, 4], f32, tag="stok")
                for i in range(4):
                    diag_stok(S_ps, stok[:, i:i + 1], i * 128)
                for tb in range(qt * 4, qt * 4 + 4):
                    ps_q = ps_a.tile([128, 512], f32, tag="mm")
                    ps_kv = ps_a.tile([128, 512], f32, tag="mm")
                    for ki in range(KT_D):
                        st, sp = (ki == 0), (ki == KT_D - 1)
                        lhs = hT[:, ki, tb * 128:(tb + 1) * 128]
                        nc.tensor.matmul(ps_q[:], lhs, wq_c[0][:, ki, :], start=st, stop=sp)
                        nc.tensor.matmul(ps_kv[:], lhs, wq_c[1][:, ki, :], start=st, stop=sp)
                    t6 = tiny_p.tile([128, 18], f32, tag="t6")
                    rqi = headnorm(ps_q, 0, NQH, t6, 0, "m4")
                    for h in range(NQH):
                        nc.vector.tensor_scalar_mul(q_tok[:, tb, h * HD:(h + 1) * HD],
                                                    ps_q[:, h * HD:(h + 1) * HD],
                                                    rqi[:, h:h + 1])
                    rki = headnorm(ps_kv, 0, NKV, t6, 12, "m2")
                    for h in range(NKV):
                        nc.vector.tensor_scalar_mul(k_tok[:, tb, h * HD:(h + 1) * HD],
                                                    ps_kv[:, h * HD:(h + 1) * HD],
                                                    rki[:, h:h + 1])
                    # v evict with ln1 scale (per-token = per-partition here)
                    nc.vector.tensor_scalar_mul(v_sb[:, tb, :], ps_kv[:, 256:512],
                                                stok[:, tb - qt * 4:tb - qt * 4 + 1])
                    # rope q and k in place
                    rotq = mis_p.tile([128, NQH, HD], f32, tag="m4")
                    rope_inplace(q_tok[:, tb, :].rearrange("p (h d) -> p h d", h=NQH),
                                 rotq, cos_sb[:, tb, :], sin_sb[:, tb, :], NQH)
                    rotk = mis_p.tile([128, NKV, HD], f32, tag="m2")
                    rope_inplace(k_tok[:, tb, :].rearrange("p (h d) -> p h d", h=NKV),
                                 rotk, cos_sb[:, tb, :], sin_sb[:, tb, :], NKV)

            # kv cache outputs
            for kv in range(NKV):
                nc.sync.dma_start(out=tok_view(pk_out[l, kv]),
                                  in_=k_tok[:, :, kv * HD:(kv + 1) * HD])
                nc.sync.dma_start(out=tok_view(pv_out[l, kv]),
                                  in_=v_sb[:, :, kv * HD:(kv + 1) * HD])

            # ---------- transposes to feature-major
            QT = qb_p.tile([128, NQH, S], f32r, tag="qb")
            KT = ktp_p.tile([128, NKV, S], f32r)
            for h in range(NQH):
                for tb in range(TB):
                    tps = ps_a.tile([128, 128], f32, tag="mm")
                    nc.tensor.transpose(tps[:], q_tok[:, tb, h * HD:(h + 1) * HD], ident)
                    nc.vector.tensor_copy(out=QT[:, h, tb * 128:(tb + 1) * 128], in_=tps[:])
            for h in range(NKV):
                for tb in range(TB):
                    tps = ps_a.tile([128, 128], f32, tag="mm")
                    nc.tensor.transpose(tps[:], k_tok[:, tb, h * HD:(h + 1) * HD], ident)
                    nc.vector.tensor_copy(out=KT[:, h, tb * 128:(tb + 1) * 128], in_=tps[:])

            # ---------- attention (scores^T layout, causal skip)
            oT = ot_p.tile([128, NQH, S], f32r, tag="ot")
            for kv in range(NKV if CFG_ATTN else 0):
                for hh in range(2):
                    qh = kv * 2 + hh
                    for qt in range(2):
                        o_ps = ps_o.tile([128, 512], f32, tag="o")
                        dn_ps = ps_r.tile([1, 512], f32, tag="row")
                        nkb = qt * 4 + 4
                        for kb in range(nkb):
                            off = max(0, kb * 128 - qt * 512)
                            w = 512 - off
                            sc = ps_a.tile([128, 512], f32, tag="mm")
                            nc.tensor.matmul(sc[:, :w], KT[:, kv, kb * 128:(kb + 1) * 128],
                                             QT[:, qh, qt * 512 + off:(qt + 1) * 512],
                                             start=True, stop=True)
                            ex = expp.tile([128, 512], f32r, tag="exp")
                            nc.scalar.activation(out=ex[:, :w], in_=sc[:, :w],
                                                 func=AF.Exp, scale=SCALE)
                            if kb * 128 >= qt * 512:  # diagonal block
                                nc.vector.tensor_mul(ex[:, 0:128], ex[:, 0:128], tri)
                            st, sp = (kb == 0), (kb == nkb - 1)
                            nc.tensor.matmul(o_ps[:, off:512],
                                             v_sb[:, kb, kv * HD:(kv + 1) * HD],
                                             ex[:, :w], start=st, stop=sp)
                            nc.tensor.matmul(dn_ps[:, off:512], ones_col,
                                             ex[:, :w], start=st, stop=sp)
                        dinv = rowp.tile([1, 512], f32r, tag="dinv")
                        nc.vector.reciprocal(out=dinv[:], in_=dn_ps[:])
                        bc = ps_a.tile([128, 512], f32, tag="mm")
                        nc.tensor.matmul(bc[:], ones_row, dinv[:], start=True, stop=True)
                        dful = mis_p.tile([128, 512], f32, tag="d5")
                        nc.scalar.copy(out=dful[:], in_=bc[:])
                        nc.vector.tensor_mul(oT[:, qh, qt * 512:(qt + 1) * 512],
                                             o_ps[:], dful[:])

            # ---------- o_proj + AllReduce
            wo_c = []
            for c in range(2):
                w = wpool.tile([128, NQH, 512], f32r, tag="w")
                nc.sync.dma_start(out=w, in_=kt_view(wo_in[l])[:, :, c * 512:(c + 1) * 512])
                wo_c.append(w)
            for qt in range(2):
                def oproj_dm(dm, qt=qt):
                    pp = ps_a.tile([128, 512], f32, tag="mm")
                    for ki in range(NQH):
                        nc.tensor.matmul(pp[:], wo_c[dm // 4][:, ki, (dm % 4) * 128:(dm % 4 + 1) * 128],
                                         oT[:, ki, qt * 512:(qt + 1) * 512],
                                         start=(ki == 0), stop=(ki == NQH - 1))
                    return pp
                allreduce_half(oproj_dm, qt)

            # ---------- MLP (ln2 applied to nT2; gate/up/down GEMMs)
            wgu_c = []
            for c in range(3):
                w = wpool.tile([128, KT_D, 512], f32r, tag="w")
                nc.sync.dma_start(out=w, in_=kt_view(wgu_in[l])[:, :, c * 512:(c + 1) * 512])
                wgu_c.append(w)
            wd_c = []
            for c in range(2):
                w = wpool.tile([128, NJ, 512], f32r, tag="w")
                nc.sync.dma_start(out=w, in_=kt_view(wd_in[l])[:, :, c * 512:(c + 1) * 512])
                wd_c.append(w)
            nT2 = ot_p.tile([128, KT_D, S], f32r, tag="ot")
            m_sb = qb_p.tile([128, NJ, S], f32r, tag="qb")
            for qt in range(2):
                t0 = qt * 512
                S_ps = ln_scale(qt)
                for ki in range(KT_D):
                    nc.vector.tensor_mul(nT2[:, ki, t0:t0 + 512],
                                         hT[:, ki, t0:t0 + 512], S_ps[:])
            # wgu is host-interleaved: chunk j//2 holds [gate_j, up_j, gate_j+1, up_j+1]
            for j in range(NJ):
                gc = (2 * (j % 2)) * 128
                uc = (2 * (j % 2) + 1) * 128
                for qt in range(2):
                    t0 = qt * 512
                    gp = ps_a.tile([128, 512], f32, tag="mm")
                    for ki in range(KT_D):
                        nc.tensor.matmul(gp[:], wgu_c[j // 2][:, ki, gc:gc + 128],
                                         nT2[:, ki, t0:t0 + 512],
                                         start=(ki == 0), stop=(ki == KT_D - 1))
                    g_t = mis_p.tile([128, 512], f32, tag="d5")
                    nc.scalar.activation(out=g_t[:], in_=gp[:], func=AF.Silu)
                    up = ps_a.tile([128, 512], f32, tag="mm")
                    for ki in range(KT_D):
                        nc.tensor.matmul(up[:], wgu_c[j // 2][:, ki, uc:uc + 128],
                                         nT2[:, ki, t0:t0 + 512],
                                         start=(ki == 0), stop=(ki == KT_D - 1))
                    nc.vector.tensor_mul(m_sb[:, j, t0:t0 + 512], up[:], g_t[:])
            for qt in range(2):
                def down_dm(dm, qt=qt):
                    pp = ps_a.tile([128, 512], f32, tag="mm")
                    for ki in range(NJ):
                        nc.tensor.matmul(pp[:], wd_c[dm // 4][:, ki, (dm % 4) * 128:(dm % 4 + 1) * 128],
                                         m_sb[:, ki, qt * 512:(qt + 1) * 512],
                                         start=(ki == 0), stop=(ki == NJ - 1))
                    return pp
                allreduce_half(down_dm, qt)

        # ---------- final norm (folded into logit eviction) + lm head
        stokf = tiny_p.tile([128, TB], f32, tag="stokf")
        for qt in range(2):
            S_ps = ln_scale(qt)
            for i in range(4):
                diag_stok(S_ps, stokf[:, qt * 4 + i:qt * 4 + i + 1], i * 128)
        for vc in range(NVC if CFG_LM else 0):
            lw = wpool.tile([128, KT_D, VC], f32r, tag="w")
            nc.sync.dma_start(out=lw, in_=kt_view(lmw_in[:])[:, :, vc * VC:(vc + 1) * VC])
            for tb in range(TB):
                pp = ps_a.tile([128, 512], f32, tag="mm")
                for ki in range(KT_D):
                    nc.tensor.matmul(pp[:, :VC], hT[:, ki, tb * 128:(tb + 1) * 128],
                                     lw[:, ki, :], start=(ki == 0), stop=(ki == KT_D - 1))
                ev = evp.tile([128, 512], f32, tag="ev")
                nc.vector.tensor_scalar_mul(ev[:, :VC], pp[:, :VC], stokf[:, tb:tb + 1])
                nc.sync.dma_start(out=logits_out[:, tb, vc * VC:(vc + 1) * VC],
                                  in_=ev[:, :VC])

    nc.compile()
    return nc


def _get_program():
    global _PROGRAM
    if _PROGRAM is None:
        _PROGRAM = _build_program()
    return _PROGRAM


def _host_prep(inputs):
    """Build the 8 per-core input maps from the full inputs."""
    pos = np.asarray(inputs["position_ids"], np.float32)  # [B, S]
    inv_freq = 1.0 / (THETA ** (np.arange(0, HD, 2, dtype=np.float32) / HD))
    in_maps = []
    for c in range(NCORES):
        dp, tp = c // TP, c % TP
        freqs = pos[dp][:, None] * inv_freq[None, :]        # [S, 64]
        emb = np.concatenate([freqs, freqs], axis=1)        # [S, 128]
        wq = np.asarray(inputs["Wq"])[:, :, tp * NQH * HD:(tp + 1) * NQH * HD]
        wk = np.asarray(inputs["Wk"])[:, :, tp * NKV * HD:(tp + 1) * NKV * HD]
        wv = np.asarray(inputs["Wv"])[:, :, tp * NKV * HD:(tp + 1) * NKV * HD]
        # fold ln weights into the in-projections (all-ones here, but general)
        ln1 = np.asarray(inputs["ln1_w"])[:, :, None]       # [L, D, 1]
        ln2 = np.asarray(inputs["ln2_w"])[:, :, None]
        lnf = np.asarray(inputs["norm_w"])[:, None]         # [D, 1]
        wqkv = np.concatenate([wq, wk, wv], axis=2) * ln1   # [L, D, 1024]
        in_maps.append({
            "x": np.ascontiguousarray(np.asarray(inputs["input_embeds"])[dp].T),
            "cos": np.cos(emb).astype(np.float32),
            "sin": np.sin(emb).astype(np.float32),
            "wqkv": np.ascontiguousarray(wqkv, np.float32),
            "wo": np.ascontiguousarray(
                np.asarray(inputs["Wo"])[:, tp * NQH * HD:(tp + 1) * NQH * HD, :], np.float32),
            "wgu": np.ascontiguousarray((np.stack(
                [np.asarray(inputs["Wgate"])[:, :, tp * FFS:(tp + 1) * FFS]
                   .reshape(L, D, NJ, 128),
                 np.asarray(inputs["Wup"])[:, :, tp * FFS:(tp + 1) * FFS]
                   .reshape(L, D, NJ, 128)],
                axis=3).reshape(L, D, 2 * FFS)) * ln2, np.float32),
            "wd": np.ascontiguousarray(
                np.asarray(inputs["Wdown"])[:, tp * FFS:(tp + 1) * FFS, :], np.float32),
            "lmw": np.ascontiguousarray(
                (np.asarray(inputs["lm_head_w"]) * lnf)[:, tp * VS:(tp + 1) * VS], np.float32),
        })
    return in_maps


def _assemble(results):
    logits = np.empty((B, S, V), np.float32)
    pk = np.empty((L, B, HK, S, HD), np.float32)
    pv = np.empty((L, B, HK, S, HD), np.float32)
    for c in range(NCORES):
        dp, tp = c // TP, c % TP
        r = results[c]
        lg = r["logits"]  # [128, TB, VS]
        logits[dp, :, tp * VS:(tp + 1) * VS] = lg.transpose(1, 0, 2).reshape(S, VS)
        pk[:, dp, tp * NKV:(tp + 1) * NKV] = r["pk"]
        pv[:, dp, tp * NKV:(tp + 1) * NKV] = r["pv"]
    return logits, pk, pv


def kernel(**inputs):
    from concourse.bass_utils import run_bass_kernel_spmd
    nc = _get_program()
    in_maps = _host_prep(inputs)
    res = run_bass_kernel_spmd(nc, in_maps, core_ids=list(range(NCORES)))
    return _assemble(res.results)
